# revision 24
# baseline (speedup 1.0000x reference)
"""Trainium2 Bass kernel for a small dense transformer block.

Module (hardcoded shapes): B=4, T=2048, D=64, H=8, FF=256.
  q/k/v: per-head full-width linears (H, D, D) + bias
  scores = q @ k.T (unscaled), causal, softmax
  out = attn @ v, concat heads -> proj (H*D -> D) + bias
  h1 = LN(x + attn_out); y = LN(h1 + relu(h1@W1+b1)@W2+b2)

Sharding: one head per core (8 heads / 8 cores). Each core computes its
head's attention and the partial projection attn_h @ (x @ Wv_h @ Wp_h);
per-batch ReduceScatters (bf16) sum partials over cores and shard
tokens; a pipelined per-batch epilogue (LN/FFN) finishes each shard.

Math folding (host-side):
  scores[t,s] = (x_t Wq + bq)·(x_s Wk + bk). Terms depending only on t
  cancel in softmax over s, so with G = Wq Wk^T and c = Wk @ bq:
    scores'[t,s] = x_t G x_s^T + c·x_s
  -> k-side projection kG = [G x; c·x] (65 rows), q-side = raw x with a
  ones row (xT, built on host, bf16). The q projection disappears.
  softmax rows sum to 1 => v/proj biases fold to the constant
  C = sum_h bv_h @ Wp_h + bp, added as C/8 per core in the drain.
  V gets a ones column so PV also produces the softmax denominator.
  LN sign trick: dst = (mu - z)*rstd*(-g) + b so the subtract order
  matches scalar_tensor_tensor's (scalar op in0) op1 in1 form.
"""

import numpy as np

B, T, D, H, FF = 4, 2048, 64, 8, 256
NTOK = B * T          # 8192
SHARD = NTOK // 8     # 1024
TB = 512              # query block
EPS = 1e-5
F32 = np.float32

_CACHE = {}


def _build_nc(single=False):
    import concourse.bass as bass
    import concourse.tile as tile
    from concourse import bacc, mybir

    f32 = mybir.dt.float32
    bf16 = mybir.dt.bfloat16
    Act = mybir.ActivationFunctionType
    Alu = mybir.AluOpType

    nc = bacc.Bacc("TRN2", target_bir_lowering=False, debug=False, num_devices=8)

    # ---- I/O ----
    xT_d = nc.dram_tensor("xT", [D + 1, NTOK], bf16, kind="ExternalInput")
    xs_d = nc.dram_tensor("xs", [SHARD, D], f32, kind="ExternalInput")
    wkg_d = nc.dram_tensor("wkg", [D, D + 1], bf16, kind="ExternalInput")
    wvv_d = nc.dram_tensor("wvv", [D + 1, D + 1], bf16, kind="ExternalInput")
    w1a_d = nc.dram_tensor("w1a", [D + 1, FF], bf16, kind="ExternalInput")
    w2_d = nc.dram_tensor("w2", [FF, D], bf16, kind="ExternalInput")
    tri_d = nc.dram_tensor("tri", [128, 128], bf16, kind="ExternalInput")
    identb_d = nc.dram_tensor("identb", [128, 128], bf16, kind="ExternalInput")
    one128_d = nc.dram_tensor("one128", [1, 128], bf16, kind="ExternalInput")
    b2r_d = nc.dram_tensor("b2r", [1, D], bf16, kind="ExternalInput")
    # broadcast constants, pre-replicated to 128 partitions on host
    c8bc_d = nc.dram_tensor("c8bc", [128, D], f32, kind="ExternalInput")
    g1bc_d = nc.dram_tensor("g1bc", [128, D], f32, kind="ExternalInput")
    be1bc_d = nc.dram_tensor("be1bc", [128, D], f32, kind="ExternalInput")
    g2bc_d = nc.dram_tensor("g2bc", [128, D], f32, kind="ExternalInput")
    be2bc_d = nc.dram_tensor("be2bc", [128, D], f32, kind="ExternalInput")
    out_d = nc.dram_tensor("out", [SHARD, D], f32, kind="ExternalOutput")

    NCHB = T // 128       # 16 key chunks per batch
    NJB = T // TB         # 4 query blocks per batch
    QS = SHARD // B       # 256 tokens per epilogue stage

    with tile.TileContext(nc) as tc:
        with (
            tc.tile_pool(name="singles", bufs=1) as singles,
            tc.tile_pool(name="work", bufs=3) as work,
            tc.tile_pool(name="drn", bufs=2) as drn,
            tc.tile_pool(name="ep", bufs=2) as ep,
            tc.tile_pool(name="scs", bufs=2, space="PSUM") as scs,
            tc.tile_pool(name="plong", bufs=2, space="PSUM") as plong,
            tc.tile_pool(name="psm", bufs=2, space="PSUM") as psm,
            tc.tile_pool(name="dram", bufs=1, space="DRAM") as dram,
        ):
            # ---- persistent SBUF ----
            xT = singles.tile([D + 1, NTOK], bf16)
            kT = singles.tile([D + 1, NTOK], bf16)
            v2 = singles.tile([128, NTOK // 128, D + 1], bf16)
            tri = singles.tile([128, 128], bf16)
            identb = singles.tile([128, 128], bf16)
            one128 = singles.tile([1, 128], bf16)
            b2r = singles.tile([1, D], bf16)
            wkg = singles.tile([D, D + 1], bf16)
            wvv = singles.tile([D + 1, D + 1], bf16)
            w1a = singles.tile([D + 1, FF], bf16)
            w2 = singles.tile([128, 2, D], bf16)
            c8bc = singles.tile([128, D], f32)
            g1bc = singles.tile([128, D], f32)
            be1bc = singles.tile([128, D], f32)
            g2bc = singles.tile([128, D], f32)
            be2bc = singles.tile([128, D], f32)
            epst = singles.tile([128, 1], f32)
            xs_all = singles.tile([128, SHARD // 128, D], f32)
            h1b = singles.tile([128, SHARD // 128, D], bf16)
            h1T = singles.tile([D + 1, SHARD], bf16)

            rs_all = dram.tile([NTOK, D], bf16, tag="rs_all", name="rs_all")
            rs_out_a = dram.tile([512, D], bf16, tag="rs_out_a", name="rs_out_a")
            rs_out_b = dram.tile([256, D], bf16, tag="rs_out_b", name="rs_out_b")
            rs_out_c = dram.tile([128, D], bf16, tag="rs_out_c", name="rs_out_c")
            rs_out_d = dram.tile([128, D], bf16, tag="rs_out_d", name="rs_out_d")
            # weights via gpsimd SWDGE (cheap dispatch); bulk x via SP HWDGE
            nc.gpsimd.dma_start(wkg[:], wkg_d[:])
            nc.gpsimd.dma_start(wvv[:], wvv_d[:])
            nc.gpsimd.dma_start(tri[:], tri_d[:])
            nc.gpsimd.dma_start(identb[:], identb_d[:])
            nc.gpsimd.dma_start(one128[:], one128_d[:])
            nc.gpsimd.dma_start(b2r[:], b2r_d[:])
            nc.gpsimd.dma_start(w1a[:], w1a_d[:])
            nc.gpsimd.dma_start(w2[:], w2_d.rearrange("(c p) d -> p c d", p=128))
            nc.gpsimd.dma_start(c8bc[:], c8bc_d[:])
            nc.gpsimd.dma_start(g1bc[:], g1bc_d[:])
            nc.gpsimd.dma_start(be1bc[:], be1bc_d[:])
            nc.gpsimd.dma_start(g2bc[:], g2bc_d[:])
            nc.gpsimd.dma_start(be2bc[:], be2bc_d[:])
            nc.vector.memset(epst[:], EPS)
            nc.vector.memset(h1T[D : D + 1, :], 1.0)
            for b in range(B):
                nc.sync.dma_start(xT[:, T * b : T * (b + 1)],
                                  xT_d[:, T * b : T * (b + 1)])
            nc.sync.dma_start(xs_all[:], xs_d.rearrange("(q p) d -> p q d", p=128))

            def emit_kg(b, i):
                """kT[:, b*T + 512*i : +512] = (wkg.T @ xT-slice), bf16."""
                t0 = b * T + TB * i
                pk = psm.tile([D + 1, TB], f32, tag="small")
                nc.tensor.matmul(pk[:], lhsT=wkg[:],
                                 rhs=xT[:D, t0 : t0 + TB],
                                 start=True, stop=True)
                nc.vector.tensor_copy(kT[:, t0 : t0 + TB], pk[:])

            def emit_v2(b, i):
                """v2 chunks 4i..4i+3 of batch b."""
                pv = psm.tile([128, 4, D + 1], f32, tag="small")
                for u in range(4):
                    ci = 16 * b + 4 * i + u
                    nc.tensor.matmul(pv[:, u, :],
                                     lhsT=xT[:, 128 * ci : 128 * (ci + 1)],
                                     rhs=wvv[:], start=True, stop=True)
                nc.vector.tensor_copy(
                    v2[:, 16 * b + 4 * i : 16 * b + 4 * (i + 1), :], pv[:])

            def emit_jblock(b, j):
                base = b * T
                t0 = base + TB * j
                nchunks = 4 * (j + 1)
                ngroups = nchunks // 2
                outT = plong.tile([D + 1, TB], f32, tag="acc")

                def c_off(c):
                    o = 128 * c - TB * j
                    return o if o > 0 else 0

                # score matmuls for group g: chunks (2g, 2g+1)
                def emit_scores(g):
                    st = scs.tile([128, 2, TB], f32, tag="sT")
                    for u in range(2):
                        c = 2 * g + u
                        o = c_off(c)
                        s0 = base + 128 * c
                        nc.tensor.matmul(
                            st[:, u, o:TB],
                            lhsT=kT[:, s0 : s0 + 128],
                            rhs=xT[:, t0 + o : t0 + TB],
                            start=True, stop=True)
                    return st

                sts = {0: emit_scores(0)}
                for g in range(ngroups):
                    if g + 1 < ngroups:
                        sts[g + 1] = emit_scores(g + 1)
                    st = sts.pop(g)
                    om = c_off(2 * g)  # min offset of the two chunks
                    ex = work.tile([128, 2, TB], bf16, tag="exp")
                    nc.scalar.activation(ex[:, :, om:TB], st[:, :, om:TB], Act.Exp)
                    for u in range(2):
                        c = 2 * g + u
                        o = c_off(c)
                        if c >= 4 * j:  # diagonal chunk: mask its 128-col edge
                            nc.vector.tensor_mul(
                                ex[:, u, o : o + 128], ex[:, u, o : o + 128], tri[:])
                        nc.tensor.matmul(
                            outT[:, o:TB],
                            lhsT=v2[:, 16 * b + c, :],
                            rhs=ex[:, u, o:TB],
                            start=(c == 0), stop=(c == nchunks - 1))

                # drain: normalize + transpose to [t, d], ship to rs_all
                oc = drn.tile([D + 1, TB], bf16, tag="oc")
                nc.vector.tensor_copy(oc[:], outT[:])
                tp = psm.tile([128, 4, D + 2], bf16, tag="small")
                for u in range(4):
                    nc.tensor.transpose(
                        tp[:, u, : D + 1], oc[:, 128 * u : 128 * (u + 1)],
                        identb[: D + 1, : D + 1])
                denf = drn.tile([128, 4, 1], f32, tag="denf")
                nc.vector.tensor_copy(denf[:], tp[:, :, D : D + 1])
                recb = drn.tile([128, 4, 1], f32, tag="rec")
                nc.vector.reciprocal_approx_fast(recb[:], denf[:])
                part = drn.tile([128, 4, D], bf16, tag="part")
                nc.vector.tensor_tensor(
                    part[:], tp[:, :, :D],
                    recb.to_broadcast((128, 4, D)), Alu.mult)
                nc.sync.dma_start(
                    rs_all[base + TB * j : base + TB * (j + 1), :]
                    .rearrange("(u p) d -> p u d", p=128),
                    part[:])

            def emit_rs(lo, hi, outt):
                seg = rs_all[lo:hi, :]
                if single:
                    n = outt.shape[0]
                    nc.sync.dma_start(outt[:], seg[:n, :])
                else:
                    nc.gpsimd.collective_compute(
                        "ReduceScatter", Alu.add,
                        replica_groups=[list(range(8))],
                        ins=[seg], outs=[outt[:]])

            def emit_stage_block(q0, nq, rsb):
                """Epilogue for nq*128 tokens (shard rows 128*q0 onward)."""
                def ln(zin, dst, g, be):
                    mt = ep.tile([128, nq, 1], f32, tag="mt")
                    nc.vector.tensor_reduce(mt[:], zin[:], mybir.AxisListType.X,
                                            Alu.add)
                    zc = ep.tile([128, nq, D], f32, tag="zc")
                    # zc = mu - z  (sign folded into g on host)
                    nc.vector.scalar_tensor_tensor(
                        zc[:], mt.to_broadcast(zin.shape), 1.0 / D, zin[:],
                        Alu.mult, Alu.subtract)
                    sq = ep.tile([128, nq, D], f32, tag="sq")
                    nc.vector.tensor_mul(sq[:], zc[:], zc[:])
                    vt = ep.tile([128, nq, 1], f32, tag="vt")
                    nc.vector.tensor_reduce(vt[:], sq[:], mybir.AxisListType.X,
                                            Alu.add)
                    sd = ep.tile([128, nq, 1], f32, tag="sd")
                    nc.scalar.activation(sd[:, :, 0], vt[:, :, 0], Act.Sqrt,
                                         bias=epst[:], scale=1.0 / D)
                    rc = ep.tile([128, nq, 1], f32, tag="rc")
                    nc.vector.reciprocal_approx_fast(rc[:], sd[:])
                    nc.vector.tensor_tensor(
                        zc[:], zc[:], rc.to_broadcast(zc.shape), Alu.mult)
                    nc.vector.tensor_tensor(
                        zc[:], zc[:], g[:, None, :].to_broadcast(zc.shape),
                        Alu.mult)
                    nc.vector.tensor_tensor(
                        dst[:], zc[:], be[:, None, :].to_broadcast(zc.shape),
                        Alu.add)

                rtt = ep.tile([128, nq, D], bf16, tag="rt")
                nc.sync.dma_start(
                    rtt[:], rsb[:].rearrange("(q p) d -> p q d", p=128))
                zt = ep.tile([128, nq, D], f32, tag="zt")
                nc.vector.tensor_tensor(
                    zt[:], xs_all[:, q0 : q0 + nq, :], rtt[:], Alu.add)
                nc.vector.tensor_tensor(
                    zt[:], zt[:], c8bc[:, None, :].to_broadcast(zt.shape),
                    Alu.add)
                h1s = h1b[:, q0 : q0 + nq, :]
                ln(zt, h1s, g1bc, be1bc)
                # h1T slice via PE transposes
                tpE = psm.tile([D, nq, 128], bf16, tag="small")
                for q in range(nq):
                    nc.tensor.transpose(tpE[:, q, :], h1s[:, q, :], identb[:])
                nc.vector.tensor_copy(
                    h1T[:D, 128 * q0 : 128 * (q0 + nq)]
                    .rearrange("p (a c) -> p a c", a=nq), tpE[:])
                # FFN up + relu (relu is resident in every act table)
                f1 = ep.tile([128, 2, nq * 128], bf16, tag="f1")
                for fc in range(2):
                    for s0 in range(0, nq * 128, 512):
                        sw = min(512, nq * 128 - s0)
                        up = psm.tile([128, 512], f32, tag="small")
                        nc.tensor.matmul(
                            up[:, :sw],
                            lhsT=w1a[:, 128 * fc : 128 * (fc + 1)],
                            rhs=h1T[:, 128 * q0 + s0 : 128 * q0 + s0 + sw],
                            start=True, stop=True)
                        nc.scalar.activation(
                            f1[:, fc, s0 : s0 + sw], up[:, :sw], Act.Relu)
                # FFN down into psum: b2 + h1 + relu(h1W1+b1)W2, then LN2
                dn = psm.tile([128, nq, D], f32, tag="small")
                for q in range(nq):
                    nc.tensor.matmul(dn[:, q, :], lhsT=one128[:], rhs=b2r[:],
                                     start=True, stop=False)
                    nc.tensor.matmul(dn[:, q, :], lhsT=identb[:],
                                     rhs=h1s[:, q, :], start=False, stop=False)
                    for fc in range(2):
                        nc.tensor.matmul(
                            dn[:, q, :],
                            lhsT=f1[:, fc, 128 * q : 128 * (q + 1)],
                            rhs=w2[:, fc, :],
                            start=False, stop=(fc == 1))
                o_st = ep.tile([128, nq, D], f32, tag="ot")
                ln(dn, o_st, g2bc, be2bc)
                nc.sync.dma_start(
                    out_d[128 * q0 : 128 * (q0 + nq), :]
                    .rearrange("(q p) d -> p q d", p=128),
                    o_st[:])

            # ---- schedule ----
            # Attention first; RS_A (batches 0-2) fires after batch 2, RS_B
            # (batch 3) at the end. All epilogue stages are emitted after the
            # attention so RS-gated DMAs never block engine queues mid-run.
            for i in range(NJB):
                emit_kg(0, i)
                emit_v2(0, i)
            for b in range(B):
                for j in range(NJB):
                    emit_jblock(b, j)
                    if b == 3 and j == 1:
                        emit_rs(6144, 7168, rs_out_c)
                    if b + 1 < B:
                        if j == 0:
                            emit_kg(b + 1, 0), emit_kg(b + 1, 1)
                        elif j == 1:
                            emit_kg(b + 1, 2), emit_kg(b + 1, 3)
                        elif j == 2:
                            emit_v2(b + 1, 0), emit_v2(b + 1, 1)
                        else:
                            emit_v2(b + 1, 2), emit_v2(b + 1, 3)
                if b == 1:
                    emit_rs(0, 4096, rs_out_a)
                elif b == 2:
                    emit_rs(4096, 6144, rs_out_b)
                elif b == 3:
                    emit_rs(7168, 8192, rs_out_d)
            emit_stage_block(0, 4, rs_out_a)
            emit_stage_block(4, 2, rs_out_b)
            emit_stage_block(6, 1, rs_out_c)
            emit_stage_block(7, 1, rs_out_d)

    nc.compile()
    return nc


def _prep_inputs(inputs, Wq, bq, Wk, bk, Wv, bv, Wp, bp, W1, b1, W2, b2,
                 g1, be1, g2, be2):
    """Host-side input prep: folded per-head weights + per-core maps."""
    import ml_dtypes

    BF16 = ml_dtypes.bfloat16
    x = np.ascontiguousarray(np.asarray(inputs, dtype=F32).reshape(NTOK, D))
    Wq, bq = np.asarray(Wq, np.float64), np.asarray(bq, np.float64)
    Wk, bk = np.asarray(Wk, np.float64), np.asarray(bk, np.float64)
    Wv, bv = np.asarray(Wv, np.float64), np.asarray(bv, np.float64)
    Wp, bp = np.asarray(Wp, np.float64), np.asarray(bp, np.float64)

    bc = lambda v: np.ascontiguousarray(
        np.broadcast_to(np.asarray(v, F32).reshape(1, D), (128, D)))
    bcb = lambda a: np.ascontiguousarray(np.asarray(a, F32).astype(BF16))
    tri = np.triu(np.ones((128, 128), F32)).astype(BF16)
    identb = np.eye(128, dtype=F32).astype(BF16)

    xTa = np.concatenate([x.T, np.ones((1, NTOK), F32)], axis=0).astype(BF16)
    xTa = np.ascontiguousarray(xTa)

    C = sum(bv[h] @ Wp[D * h : D * (h + 1)] for h in range(H)) + bp

    common = dict(
        xT=xTa, tri=tri, identb=identb,
        one128=np.ones((1, 128), F32).astype(BF16),
        b2r=bcb(np.asarray(b2, F32).reshape(1, D)),
        w1a=bcb(np.concatenate(
            [np.asarray(W1, F32), np.asarray(b1, F32).reshape(1, FF)], axis=0)),
        w2=bcb(np.asarray(W2, F32)),
        c8bc=bc(C.astype(F32)),
        g1bc=bc(-np.asarray(g1, F32)), be1bc=bc(be1),
        g2bc=bc(-np.asarray(g2, F32)), be2bc=bc(be2),
    )

    in_maps = []
    for h in range(H):
        # kG weights: wkg[d, r<64] = (Wq Wk^T)[r, d]; wkg[:, 64] = Wk @ bq
        G = Wq[h] @ Wk[h].T
        c = Wk[h] @ bq[h]
        wkg = np.concatenate([G.T, c.reshape(D, 1)], axis=1)  # [64, 65]
        # V path: wvv[:64, :64] = Wv @ Wp_h; ones column via xT ones row
        wvp = Wv[h] @ Wp[D * h : D * (h + 1)]
        wvv = np.zeros((D + 1, D + 1), np.float64)
        wvv[:D, :D] = wvp
        wvv[D, D] = 1.0
        # this core's token shard, per RS segment
        xs_h = np.concatenate(
            [x[512 * h : 512 * (h + 1)],
             x[4096 + 256 * h : 4096 + 256 * (h + 1)],
             x[6144 + 128 * h : 6144 + 128 * (h + 1)],
             x[7168 + 128 * h : 7168 + 128 * (h + 1)]])
        in_maps.append(dict(
            common,
            xs=np.ascontiguousarray(xs_h),
            wkg=np.ascontiguousarray(wkg.astype(F32).astype(BF16)),
            wvv=np.ascontiguousarray(wvv.astype(F32).astype(BF16)),
        ))
    return in_maps


def _gather(results) -> np.ndarray:
    """Reassemble per-core output shards into the full [NTOK, D] output."""
    out = np.empty((NTOK, D), F32)
    for c in range(8):
        shard = results[c]["out"]
        out[512 * c : 512 * (c + 1)] = shard[:512]
        out[4096 + 256 * c : 4096 + 256 * (c + 1)] = shard[512:768]
        out[6144 + 128 * c : 6144 + 128 * (c + 1)] = shard[768:896]
        out[7168 + 128 * c : 7168 + 128 * (c + 1)] = shard[896:]
    return out


def _get_nc():
    if "nc" not in _CACHE:
        _CACHE["nc"] = _build_nc()
    return _CACHE["nc"]


def kernel(**inputs) -> np.ndarray:
    from concourse.bass_utils import run_bass_kernel_spmd

    in_maps = _prep_inputs(**inputs)
    nc = _get_nc()
    res = run_bass_kernel_spmd(nc, in_maps, list(range(8)))
    return _gather(res.results).reshape(B, T, D)


# revision 25
# speedup vs baseline: 1.0306x; 1.0306x over previous
"""Trainium2 Bass kernel for a small dense transformer block.

Module (hardcoded shapes): B=4, T=2048, D=64, H=8, FF=256.
  q/k/v: per-head full-width linears (H, D, D) + bias
  scores = q @ k.T (unscaled), causal, softmax
  out = attn @ v, concat heads -> proj (H*D -> D) + bias
  h1 = LN(x + attn_out); y = LN(h1 + relu(h1@W1+b1)@W2+b2)

Sharding: one head per core (8 heads / 8 cores). Each core computes its
head's attention and the partial projection attn_h @ (x @ Wv_h @ Wp_h);
per-batch ReduceScatters (bf16) sum partials over cores and shard
tokens; a pipelined per-batch epilogue (LN/FFN) finishes each shard.

Math folding (host-side):
  scores[t,s] = (x_t Wq + bq)·(x_s Wk + bk). Terms depending only on t
  cancel in softmax over s, so with G = Wq Wk^T and c = Wk @ bq:
    scores'[t,s] = x_t G x_s^T + c·x_s
  -> k-side projection kG = [G x; c·x] (65 rows), q-side = raw x with a
  ones row (xT, built on host, bf16). The q projection disappears.
  softmax rows sum to 1 => v/proj biases fold to the constant
  C = sum_h bv_h @ Wp_h + bp, added as C/8 per core in the drain.
  V gets a ones column so PV also produces the softmax denominator.
  LN sign trick: dst = (mu - z)*rstd*(-g) + b so the subtract order
  matches scalar_tensor_tensor's (scalar op in0) op1 in1 form.
"""

import numpy as np

B, T, D, H, FF = 4, 2048, 64, 8, 256
NTOK = B * T          # 8192
SHARD = NTOK // 8     # 1024
TB = 512              # query block
EPS = 1e-5
F32 = np.float32

_CACHE = {}


def _build_nc(single=False):
    import concourse.bass as bass
    import concourse.tile as tile
    from concourse import bacc, mybir

    f32 = mybir.dt.float32
    bf16 = mybir.dt.bfloat16
    Act = mybir.ActivationFunctionType
    Alu = mybir.AluOpType

    nc = bacc.Bacc("TRN2", target_bir_lowering=False, debug=False, num_devices=8)

    # ---- I/O ----
    xT_d = nc.dram_tensor("xT", [D + 1, NTOK], bf16, kind="ExternalInput")
    xs_d = nc.dram_tensor("xs", [SHARD, D], f32, kind="ExternalInput")
    wkg_d = nc.dram_tensor("wkg", [D, D + 1], bf16, kind="ExternalInput")
    wvv_d = nc.dram_tensor("wvv", [D + 1, D + 1], bf16, kind="ExternalInput")
    w1a_d = nc.dram_tensor("w1a", [D + 1, FF], bf16, kind="ExternalInput")
    w2_d = nc.dram_tensor("w2", [FF, D], bf16, kind="ExternalInput")
    tri_d = nc.dram_tensor("tri", [128, 128], bf16, kind="ExternalInput")
    identb_d = nc.dram_tensor("identb", [128, 128], bf16, kind="ExternalInput")
    one128_d = nc.dram_tensor("one128", [1, 128], bf16, kind="ExternalInput")
    b2r_d = nc.dram_tensor("b2r", [1, D], bf16, kind="ExternalInput")
    # broadcast constants, pre-replicated to 128 partitions on host
    c8bc_d = nc.dram_tensor("c8bc", [128, D], f32, kind="ExternalInput")
    g1bc_d = nc.dram_tensor("g1bc", [128, D], f32, kind="ExternalInput")
    be1bc_d = nc.dram_tensor("be1bc", [128, D], f32, kind="ExternalInput")
    g2bc_d = nc.dram_tensor("g2bc", [128, D], f32, kind="ExternalInput")
    be2bc_d = nc.dram_tensor("be2bc", [128, D], f32, kind="ExternalInput")
    out_d = nc.dram_tensor("out", [SHARD, D], f32, kind="ExternalOutput")

    NCHB = T // 128       # 16 key chunks per batch
    NJB = T // TB         # 4 query blocks per batch
    QS = SHARD // B       # 256 tokens per epilogue stage

    with tile.TileContext(nc) as tc:
        with (
            tc.tile_pool(name="singles", bufs=1) as singles,
            tc.tile_pool(name="work", bufs=3) as work,
            tc.tile_pool(name="drn", bufs=2) as drn,
            tc.tile_pool(name="ep", bufs=2) as ep,
            tc.tile_pool(name="scs", bufs=2, space="PSUM") as scs,
            tc.tile_pool(name="plong", bufs=2, space="PSUM") as plong,
            tc.tile_pool(name="psm", bufs=2, space="PSUM") as psm,
            tc.tile_pool(name="dram", bufs=1, space="DRAM") as dram,
        ):
            # ---- persistent SBUF ----
            xT = singles.tile([D + 1, NTOK], bf16)
            kT = singles.tile([D + 1, NTOK], bf16)
            v2 = singles.tile([128, NTOK // 128, D + 1], bf16)
            tri = singles.tile([128, 128], bf16)
            identb = singles.tile([128, 128], bf16)
            one128 = singles.tile([1, 128], bf16)
            b2r = singles.tile([1, D], bf16)
            wkg = singles.tile([D, D + 1], bf16)
            wvv = singles.tile([D + 1, D + 1], bf16)
            w1a = singles.tile([D + 1, FF], bf16)
            w2 = singles.tile([128, 2, D], bf16)
            c8bc = singles.tile([128, D], f32)
            g1bc = singles.tile([128, D], f32)
            be1bc = singles.tile([128, D], f32)
            g2bc = singles.tile([128, D], f32)
            be2bc = singles.tile([128, D], f32)
            epst = singles.tile([128, 1], f32)
            xs_all = singles.tile([128, SHARD // 128, D], f32)
            h1b = singles.tile([128, SHARD // 128, D], bf16)
            h1T = singles.tile([D + 1, SHARD], bf16)

            rs_a_in = dram.tile([4096, D], bf16, tag="rs_a_in", name="rs_a_in")
            rs_b_in = dram.tile([2048, D], bf16, tag="rs_b_in", name="rs_b_in")
            rs_c_in = dram.tile([1024, D], bf16, tag="rs_c_in", name="rs_c_in")
            rs_d_in = dram.tile([1024, D], bf16, tag="rs_d_in", name="rs_d_in")
            rs_out_a = dram.tile([512, D], bf16, tag="rs_out_a", name="rs_out_a")
            rs_out_b = dram.tile([256, D], bf16, tag="rs_out_b", name="rs_out_b")
            rs_out_c = dram.tile([128, D], bf16, tag="rs_out_c", name="rs_out_c")
            rs_out_d = dram.tile([128, D], bf16, tag="rs_out_d", name="rs_out_d")
            # weights via gpsimd SWDGE (cheap dispatch); bulk x via SP HWDGE
            nc.gpsimd.dma_start(wkg[:], wkg_d[:])
            nc.gpsimd.dma_start(wvv[:], wvv_d[:])
            nc.gpsimd.dma_start(tri[:], tri_d[:])
            nc.gpsimd.dma_start(identb[:], identb_d[:])
            nc.gpsimd.dma_start(one128[:], one128_d[:])
            nc.gpsimd.dma_start(b2r[:], b2r_d[:])
            nc.gpsimd.dma_start(w1a[:], w1a_d[:])
            nc.gpsimd.dma_start(w2[:], w2_d.rearrange("(c p) d -> p c d", p=128))
            nc.gpsimd.dma_start(c8bc[:], c8bc_d[:])
            nc.gpsimd.dma_start(g1bc[:], g1bc_d[:])
            nc.gpsimd.dma_start(be1bc[:], be1bc_d[:])
            nc.gpsimd.dma_start(g2bc[:], g2bc_d[:])
            nc.gpsimd.dma_start(be2bc[:], be2bc_d[:])
            nc.vector.memset(epst[:], EPS)
            nc.vector.memset(h1T[D : D + 1, :], 1.0)
            for b in range(B):
                nc.sync.dma_start(xT[:, T * b : T * (b + 1)],
                                  xT_d[:, T * b : T * (b + 1)])
            nc.sync.dma_start(xs_all[:], xs_d.rearrange("(q p) d -> p q d", p=128))

            def emit_kg(b, i):
                """kT[:, b*T + 512*i : +512] = (wkg.T @ xT-slice), bf16."""
                t0 = b * T + TB * i
                pk = psm.tile([D + 1, TB], f32, tag="small")
                nc.tensor.matmul(pk[:], lhsT=wkg[:],
                                 rhs=xT[:D, t0 : t0 + TB],
                                 start=True, stop=True)
                nc.vector.tensor_copy(kT[:, t0 : t0 + TB], pk[:])

            def emit_v2(b, i):
                """v2 chunks 4i..4i+3 of batch b."""
                pv = psm.tile([128, 4, D + 1], f32, tag="small")
                for u in range(4):
                    ci = 16 * b + 4 * i + u
                    nc.tensor.matmul(pv[:, u, :],
                                     lhsT=xT[:, 128 * ci : 128 * (ci + 1)],
                                     rhs=wvv[:], start=True, stop=True)
                nc.vector.tensor_copy(
                    v2[:, 16 * b + 4 * i : 16 * b + 4 * (i + 1), :], pv[:])

            def emit_jblock(b, j):
                base = b * T
                t0 = base + TB * j
                nchunks = 4 * (j + 1)
                ngroups = nchunks // 2
                outT = plong.tile([D + 1, TB], f32, tag="acc")

                def c_off(c):
                    o = 128 * c - TB * j
                    return o if o > 0 else 0

                # score matmuls for group g: chunks (2g, 2g+1)
                def emit_scores(g):
                    st = scs.tile([128, 2, TB], f32, tag="sT")
                    for u in range(2):
                        c = 2 * g + u
                        o = c_off(c)
                        s0 = base + 128 * c
                        nc.tensor.matmul(
                            st[:, u, o:TB],
                            lhsT=kT[:, s0 : s0 + 128],
                            rhs=xT[:, t0 + o : t0 + TB],
                            start=True, stop=True)
                    return st

                sts = {0: emit_scores(0)}
                for g in range(ngroups):
                    if g + 1 < ngroups:
                        sts[g + 1] = emit_scores(g + 1)
                    st = sts.pop(g)
                    om = c_off(2 * g)  # min offset of the two chunks
                    ex = work.tile([128, 2, TB], bf16, tag="exp")
                    nc.scalar.activation(ex[:, :, om:TB], st[:, :, om:TB], Act.Exp)
                    for u in range(2):
                        c = 2 * g + u
                        o = c_off(c)
                        if c >= 4 * j:  # diagonal chunk: mask its 128-col edge
                            nc.vector.tensor_mul(
                                ex[:, u, o : o + 128], ex[:, u, o : o + 128], tri[:])
                        nc.tensor.matmul(
                            outT[:, o:TB],
                            lhsT=v2[:, 16 * b + c, :],
                            rhs=ex[:, u, o:TB],
                            start=(c == 0), stop=(c == nchunks - 1))

                # drain: normalize + transpose to [t, d], ship to rs_all
                oc = drn.tile([D + 1, TB], bf16, tag="oc")
                nc.vector.tensor_copy(oc[:], outT[:])
                tp = psm.tile([128, 4, D + 2], bf16, tag="small")
                for u in range(4):
                    nc.tensor.transpose(
                        tp[:, u, : D + 1], oc[:, 128 * u : 128 * (u + 1)],
                        identb[: D + 1, : D + 1])
                denf = drn.tile([128, 4, 1], f32, tag="denf")
                nc.vector.tensor_copy(denf[:], tp[:, :, D : D + 1])
                recb = drn.tile([128, 4, 1], f32, tag="rec")
                nc.vector.reciprocal_approx_fast(recb[:], denf[:])
                part = drn.tile([128, 4, D], bf16, tag="part")
                nc.vector.tensor_tensor(
                    part[:], tp[:, :, :D],
                    recb.to_broadcast((128, 4, D)), Alu.mult)
                if b < 2:
                    seg, row = rs_a_in, T * b + TB * j
                elif b == 2:
                    seg, row = rs_b_in, TB * j
                elif j < 2:
                    seg, row = rs_c_in, TB * j
                else:
                    seg, row = rs_d_in, TB * (j - 2)
                nc.sync.dma_start(
                    seg[row : row + TB, :]
                    .rearrange("(u p) d -> p u d", p=128),
                    part[:])

            def emit_rs(seg, outt):
                if single:
                    n = outt.shape[0]
                    nc.sync.dma_start(outt[:], seg[:n, :])
                else:
                    nc.gpsimd.collective_compute(
                        "ReduceScatter", Alu.add,
                        replica_groups=[list(range(8))],
                        ins=[seg], outs=[outt[:]])

            def emit_stage_block(q0, nq, rsb):
                """Epilogue for nq*128 tokens (shard rows 128*q0 onward)."""
                def ln(zin, dst, g, be):
                    mt = ep.tile([128, nq, 1], f32, tag="mt")
                    nc.vector.tensor_reduce(mt[:], zin[:], mybir.AxisListType.X,
                                            Alu.add)
                    zc = ep.tile([128, nq, D], f32, tag="zc")
                    # zc = mu - z  (sign folded into g on host)
                    nc.vector.scalar_tensor_tensor(
                        zc[:], mt.to_broadcast(zin.shape), 1.0 / D, zin[:],
                        Alu.mult, Alu.subtract)
                    sq = ep.tile([128, nq, D], f32, tag="sq")
                    nc.vector.tensor_mul(sq[:], zc[:], zc[:])
                    vt = ep.tile([128, nq, 1], f32, tag="vt")
                    nc.vector.tensor_reduce(vt[:], sq[:], mybir.AxisListType.X,
                                            Alu.add)
                    sd = ep.tile([128, nq, 1], f32, tag="sd")
                    nc.scalar.activation(sd[:, :, 0], vt[:, :, 0], Act.Sqrt,
                                         bias=epst[:], scale=1.0 / D)
                    rc = ep.tile([128, nq, 1], f32, tag="rc")
                    nc.vector.reciprocal_approx_fast(rc[:], sd[:])
                    nc.vector.tensor_tensor(
                        zc[:], zc[:], rc.to_broadcast(zc.shape), Alu.mult)
                    nc.vector.tensor_tensor(
                        zc[:], zc[:], g[:, None, :].to_broadcast(zc.shape),
                        Alu.mult)
                    nc.vector.tensor_tensor(
                        dst[:], zc[:], be[:, None, :].to_broadcast(zc.shape),
                        Alu.add)

                rtt = ep.tile([128, nq, D], bf16, tag="rt")
                nc.sync.dma_start(
                    rtt[:], rsb[:].rearrange("(q p) d -> p q d", p=128))
                zt = ep.tile([128, nq, D], f32, tag="zt")
                nc.vector.tensor_tensor(
                    zt[:], xs_all[:, q0 : q0 + nq, :], rtt[:], Alu.add)
                nc.vector.tensor_tensor(
                    zt[:], zt[:], c8bc[:, None, :].to_broadcast(zt.shape),
                    Alu.add)
                h1s = h1b[:, q0 : q0 + nq, :]
                ln(zt, h1s, g1bc, be1bc)
                # h1T slice via PE transposes
                tpE = psm.tile([D, nq, 128], bf16, tag="small")
                for q in range(nq):
                    nc.tensor.transpose(tpE[:, q, :], h1s[:, q, :], identb[:])
                nc.vector.tensor_copy(
                    h1T[:D, 128 * q0 : 128 * (q0 + nq)]
                    .rearrange("p (a c) -> p a c", a=nq), tpE[:])
                # FFN up + relu (relu is resident in every act table)
                f1 = ep.tile([128, 2, nq * 128], bf16, tag="f1")
                for fc in range(2):
                    for s0 in range(0, nq * 128, 512):
                        sw = min(512, nq * 128 - s0)
                        up = psm.tile([128, 512], f32, tag="small")
                        nc.tensor.matmul(
                            up[:, :sw],
                            lhsT=w1a[:, 128 * fc : 128 * (fc + 1)],
                            rhs=h1T[:, 128 * q0 + s0 : 128 * q0 + s0 + sw],
                            start=True, stop=True)
                        nc.scalar.activation(
                            f1[:, fc, s0 : s0 + sw], up[:, :sw], Act.Relu)
                # FFN down into psum: b2 + h1 + relu(h1W1+b1)W2, then LN2
                dn = psm.tile([128, nq, D], f32, tag="small")
                for q in range(nq):
                    nc.tensor.matmul(dn[:, q, :], lhsT=one128[:], rhs=b2r[:],
                                     start=True, stop=False)
                    nc.tensor.matmul(dn[:, q, :], lhsT=identb[:],
                                     rhs=h1s[:, q, :], start=False, stop=False)
                    for fc in range(2):
                        nc.tensor.matmul(
                            dn[:, q, :],
                            lhsT=f1[:, fc, 128 * q : 128 * (q + 1)],
                            rhs=w2[:, fc, :],
                            start=False, stop=(fc == 1))
                o_st = ep.tile([128, nq, D], f32, tag="ot")
                ln(dn, o_st, g2bc, be2bc)
                nc.sync.dma_start(
                    out_d[128 * q0 : 128 * (q0 + nq), :]
                    .rearrange("(q p) d -> p q d", p=128),
                    o_st[:])

            # ---- schedule ----
            # Attention first; RS_A (batches 0-2) fires after batch 2, RS_B
            # (batch 3) at the end. All epilogue stages are emitted after the
            # attention so RS-gated DMAs never block engine queues mid-run.
            for i in range(NJB):
                emit_kg(0, i)
                emit_v2(0, i)
            for b in range(B):
                for j in range(NJB):
                    emit_jblock(b, j)
                    if b == 3 and j == 1:
                        emit_rs(rs_c_in, rs_out_c)
                    if b + 1 < B:
                        if j == 0:
                            emit_kg(b + 1, 0), emit_kg(b + 1, 1)
                        elif j == 1:
                            emit_kg(b + 1, 2), emit_kg(b + 1, 3)
                        elif j == 2:
                            emit_v2(b + 1, 0), emit_v2(b + 1, 1)
                        else:
                            emit_v2(b + 1, 2), emit_v2(b + 1, 3)
                if b == 1:
                    emit_rs(rs_a_in, rs_out_a)
                elif b == 2:
                    emit_rs(rs_b_in, rs_out_b)
                elif b == 3:
                    emit_rs(rs_d_in, rs_out_d)
            emit_stage_block(0, 4, rs_out_a)
            emit_stage_block(4, 2, rs_out_b)
            emit_stage_block(6, 1, rs_out_c)
            emit_stage_block(7, 1, rs_out_d)

    nc.compile()
    return nc


def _prep_inputs(inputs, Wq, bq, Wk, bk, Wv, bv, Wp, bp, W1, b1, W2, b2,
                 g1, be1, g2, be2):
    """Host-side input prep: folded per-head weights + per-core maps."""
    import ml_dtypes

    BF16 = ml_dtypes.bfloat16
    x = np.ascontiguousarray(np.asarray(inputs, dtype=F32).reshape(NTOK, D))
    Wq, bq = np.asarray(Wq, np.float64), np.asarray(bq, np.float64)
    Wk, bk = np.asarray(Wk, np.float64), np.asarray(bk, np.float64)
    Wv, bv = np.asarray(Wv, np.float64), np.asarray(bv, np.float64)
    Wp, bp = np.asarray(Wp, np.float64), np.asarray(bp, np.float64)

    bc = lambda v: np.ascontiguousarray(
        np.broadcast_to(np.asarray(v, F32).reshape(1, D), (128, D)))
    bcb = lambda a: np.ascontiguousarray(np.asarray(a, F32).astype(BF16))
    tri = np.triu(np.ones((128, 128), F32)).astype(BF16)
    identb = np.eye(128, dtype=F32).astype(BF16)

    xTa = np.concatenate([x.T, np.ones((1, NTOK), F32)], axis=0).astype(BF16)
    xTa = np.ascontiguousarray(xTa)

    C = sum(bv[h] @ Wp[D * h : D * (h + 1)] for h in range(H)) + bp

    common = dict(
        xT=xTa, tri=tri, identb=identb,
        one128=np.ones((1, 128), F32).astype(BF16),
        b2r=bcb(np.asarray(b2, F32).reshape(1, D)),
        w1a=bcb(np.concatenate(
            [np.asarray(W1, F32), np.asarray(b1, F32).reshape(1, FF)], axis=0)),
        w2=bcb(np.asarray(W2, F32)),
        c8bc=bc(C.astype(F32)),
        g1bc=bc(-np.asarray(g1, F32)), be1bc=bc(be1),
        g2bc=bc(-np.asarray(g2, F32)), be2bc=bc(be2),
    )

    in_maps = []
    for h in range(H):
        # kG weights: wkg[d, r<64] = (Wq Wk^T)[r, d]; wkg[:, 64] = Wk @ bq
        G = Wq[h] @ Wk[h].T
        c = Wk[h] @ bq[h]
        wkg = np.concatenate([G.T, c.reshape(D, 1)], axis=1)  # [64, 65]
        # V path: wvv[:64, :64] = Wv @ Wp_h; ones column via xT ones row
        wvp = Wv[h] @ Wp[D * h : D * (h + 1)]
        wvv = np.zeros((D + 1, D + 1), np.float64)
        wvv[:D, :D] = wvp
        wvv[D, D] = 1.0
        # this core's token shard, per RS segment
        xs_h = np.concatenate(
            [x[512 * h : 512 * (h + 1)],
             x[4096 + 256 * h : 4096 + 256 * (h + 1)],
             x[6144 + 128 * h : 6144 + 128 * (h + 1)],
             x[7168 + 128 * h : 7168 + 128 * (h + 1)]])
        in_maps.append(dict(
            common,
            xs=np.ascontiguousarray(xs_h),
            wkg=np.ascontiguousarray(wkg.astype(F32).astype(BF16)),
            wvv=np.ascontiguousarray(wvv.astype(F32).astype(BF16)),
        ))
    return in_maps


def _gather(results) -> np.ndarray:
    """Reassemble per-core output shards into the full [NTOK, D] output."""
    out = np.empty((NTOK, D), F32)
    for c in range(8):
        shard = results[c]["out"]
        out[512 * c : 512 * (c + 1)] = shard[:512]
        out[4096 + 256 * c : 4096 + 256 * (c + 1)] = shard[512:768]
        out[6144 + 128 * c : 6144 + 128 * (c + 1)] = shard[768:896]
        out[7168 + 128 * c : 7168 + 128 * (c + 1)] = shard[896:]
    return out


def _get_nc():
    if "nc" not in _CACHE:
        _CACHE["nc"] = _build_nc()
    return _CACHE["nc"]


def kernel(**inputs) -> np.ndarray:
    from concourse.bass_utils import run_bass_kernel_spmd

    in_maps = _prep_inputs(**inputs)
    nc = _get_nc()
    res = run_bass_kernel_spmd(nc, in_maps, list(range(8)))
    return _gather(res.results).reshape(B, T, D)


# revision 26
# speedup vs baseline: 1.1000x; 1.0673x over previous
"""Trainium2 Bass kernel for a small dense transformer block.

Module (hardcoded shapes): B=4, T=2048, D=64, H=8, FF=256.
  q/k/v: per-head full-width linears (H, D, D) + bias
  scores = q @ k.T (unscaled), causal, softmax
  out = attn @ v, concat heads -> proj (H*D -> D) + bias
  h1 = LN(x + attn_out); y = LN(h1 + relu(h1@W1+b1)@W2+b2)

Sharding: one head per core (8 heads / 8 cores). Each core computes its
head's attention and the partial projection attn_h @ (x @ Wv_h @ Wp_h);
per-batch ReduceScatters (bf16) sum partials over cores and shard
tokens; a pipelined per-batch epilogue (LN/FFN) finishes each shard.

Math folding (host-side):
  scores[t,s] = (x_t Wq + bq)·(x_s Wk + bk). Terms depending only on t
  cancel in softmax over s, so with G = Wq Wk^T and c = Wk @ bq:
    scores'[t,s] = x_t G x_s^T + c·x_s
  -> k-side projection kG = [G x; c·x] (65 rows), q-side = raw x with a
  ones row (xT, built on host, bf16). The q projection disappears.
  softmax rows sum to 1 => v/proj biases fold to the constant
  C = sum_h bv_h @ Wp_h + bp, added as C/8 per core in the drain.
  V gets a ones column so PV also produces the softmax denominator.
  LN sign trick: dst = (mu - z)*rstd*(-g) + b so the subtract order
  matches scalar_tensor_tensor's (scalar op in0) op1 in1 form.
"""

import numpy as np

B, T, D, H, FF = 4, 2048, 64, 8, 256
NTOK = B * T          # 8192
SHARD = NTOK // 8     # 1024
TB = 512              # query block
EPS = 1e-5
F32 = np.float32

_CACHE = {}


def _build_nc(single=False):
    import concourse.bass as bass
    import concourse.tile as tile
    from concourse import bacc, mybir

    f32 = mybir.dt.float32
    bf16 = mybir.dt.bfloat16
    Act = mybir.ActivationFunctionType
    Alu = mybir.AluOpType

    nc = bacc.Bacc("TRN2", target_bir_lowering=False, debug=False, num_devices=8)

    # ---- I/O ----
    xT_d = nc.dram_tensor("xT", [D + 1, NTOK], bf16, kind="ExternalInput")
    xs_d = nc.dram_tensor("xs", [SHARD, D], f32, kind="ExternalInput")
    wkg_d = nc.dram_tensor("wkg", [D, D + 1], bf16, kind="ExternalInput")
    wvv_d = nc.dram_tensor("wvv", [D + 1, D + 1], bf16, kind="ExternalInput")
    w1a_d = nc.dram_tensor("w1a", [D + 1, FF], bf16, kind="ExternalInput")
    w2_d = nc.dram_tensor("w2", [FF, D], bf16, kind="ExternalInput")
    tri_d = nc.dram_tensor("tri", [128, 128], bf16, kind="ExternalInput")
    identb_d = nc.dram_tensor("identb", [128, 128], bf16, kind="ExternalInput")
    one128_d = nc.dram_tensor("one128", [1, 128], bf16, kind="ExternalInput")
    b2r_d = nc.dram_tensor("b2r", [1, D], bf16, kind="ExternalInput")
    # broadcast constants, pre-replicated to 128 partitions on host
    c8bc_d = nc.dram_tensor("c8bc", [128, D], f32, kind="ExternalInput")
    g1bc_d = nc.dram_tensor("g1bc", [128, D], f32, kind="ExternalInput")
    be1bc_d = nc.dram_tensor("be1bc", [128, D], f32, kind="ExternalInput")
    g2bc_d = nc.dram_tensor("g2bc", [128, D], f32, kind="ExternalInput")
    be2bc_d = nc.dram_tensor("be2bc", [128, D], f32, kind="ExternalInput")
    out_d = nc.dram_tensor("out", [SHARD, D], f32, kind="ExternalOutput")

    NCHB = T // 128       # 16 key chunks per batch
    NJB = T // TB         # 4 query blocks per batch
    QS = SHARD // B       # 256 tokens per epilogue stage

    with tile.TileContext(nc) as tc:
        with (
            tc.tile_pool(name="singles", bufs=1) as singles,
            tc.tile_pool(name="work", bufs=3) as work,
            tc.tile_pool(name="drn", bufs=2) as drn,
            tc.tile_pool(name="ep", bufs=2) as ep,
            tc.tile_pool(name="scs", bufs=2, space="PSUM") as scs,
            tc.tile_pool(name="plong", bufs=2, space="PSUM") as plong,
            tc.tile_pool(name="psm", bufs=2, space="PSUM") as psm,
            tc.tile_pool(name="dram", bufs=1, space="DRAM") as dram,
        ):
            # ---- persistent SBUF ----
            xT = singles.tile([D + 1, NTOK], bf16)
            kT = singles.tile([D + 1, NTOK], bf16)
            v2 = singles.tile([128, NTOK // 128, D + 1], bf16)
            tri = singles.tile([128, 128], bf16)
            identb = singles.tile([128, 128], bf16)
            one128 = singles.tile([1, 128], bf16)
            b2r = singles.tile([1, D], bf16)
            wkg = singles.tile([D, D + 1], bf16)
            wvv = singles.tile([D + 1, D + 1], bf16)
            w1a = singles.tile([D + 1, FF], bf16)
            w2 = singles.tile([128, 2, D], bf16)
            c8bc = singles.tile([128, D], f32)
            g1bc = singles.tile([128, D], f32)
            be1bc = singles.tile([128, D], f32)
            g2bc = singles.tile([128, D], f32)
            be2bc = singles.tile([128, D], f32)
            epst = singles.tile([128, 1], f32)
            xs_all = singles.tile([128, SHARD // 128, D], f32)
            h1b = singles.tile([128, SHARD // 128, D], bf16)
            h1T = singles.tile([D + 1, SHARD], bf16)

            rs_a_in = dram.tile([4096, D], bf16, tag="rs_a_in", name="rs_a_in")
            rs_b_in = dram.tile([2048, D], bf16, tag="rs_b_in", name="rs_b_in")
            rs_c_in = dram.tile([1024, D], bf16, tag="rs_c_in", name="rs_c_in")
            rs_d_in = dram.tile([1024, D], bf16, tag="rs_d_in", name="rs_d_in")
            rs_out_a = dram.tile([512, D], bf16, tag="rs_out_a", name="rs_out_a")
            rs_out_b = dram.tile([256, D], bf16, tag="rs_out_b", name="rs_out_b")
            rs_out_c = dram.tile([128, D], bf16, tag="rs_out_c", name="rs_out_c")
            rs_out_d = dram.tile([128, D], bf16, tag="rs_out_d", name="rs_out_d")
            # weights via gpsimd SWDGE (cheap dispatch); bulk x via SP HWDGE
            nc.gpsimd.dma_start(wkg[:], wkg_d[:])
            nc.gpsimd.dma_start(wvv[:], wvv_d[:])
            nc.gpsimd.dma_start(tri[:], tri_d[:])
            nc.gpsimd.dma_start(identb[:], identb_d[:])
            nc.gpsimd.dma_start(one128[:], one128_d[:])
            nc.gpsimd.dma_start(b2r[:], b2r_d[:])
            nc.gpsimd.dma_start(w1a[:], w1a_d[:])
            nc.gpsimd.dma_start(w2[:], w2_d.rearrange("(c p) d -> p c d", p=128))
            nc.gpsimd.dma_start(c8bc[:], c8bc_d[:])
            nc.gpsimd.dma_start(g1bc[:], g1bc_d[:])
            nc.gpsimd.dma_start(be1bc[:], be1bc_d[:])
            nc.gpsimd.dma_start(g2bc[:], g2bc_d[:])
            nc.gpsimd.dma_start(be2bc[:], be2bc_d[:])
            nc.vector.memset(epst[:], EPS)
            nc.vector.memset(h1T[D : D + 1, :], 1.0)
            for b in range(B):
                nc.sync.dma_start(xT[:, T * b : T * (b + 1)],
                                  xT_d[:, T * b : T * (b + 1)])
            nc.sync.dma_start(xs_all[:], xs_d.rearrange("(q p) d -> p q d", p=128))

            def emit_kg(b, i):
                """kT[:, b*T + 512*i : +512] = (wkg.T @ xT-slice), bf16."""
                t0 = b * T + TB * i
                pk = psm.tile([D + 1, TB], f32, tag="small")
                nc.tensor.matmul(pk[:], lhsT=wkg[:],
                                 rhs=xT[:D, t0 : t0 + TB],
                                 start=True, stop=True)
                nc.vector.tensor_copy(kT[:, t0 : t0 + TB], pk[:])

            def emit_v2(b, i):
                """v2 chunks 4i..4i+3 of batch b."""
                pv = psm.tile([128, 4, D + 1], f32, tag="small")
                for u in range(4):
                    ci = 16 * b + 4 * i + u
                    nc.tensor.matmul(pv[:, u, :],
                                     lhsT=xT[:, 128 * ci : 128 * (ci + 1)],
                                     rhs=wvv[:], start=True, stop=True)
                nc.vector.tensor_copy(
                    v2[:, 16 * b + 4 * i : 16 * b + 4 * (i + 1), :], pv[:])

            def emit_jblock(b, j):
                base = b * T
                t0 = base + TB * j
                nchunks = 4 * (j + 1)
                ngroups = nchunks // 2
                outT = plong.tile([D + 1, TB], f32, tag="acc")

                def c_off(c):
                    o = 128 * c - TB * j
                    return o if o > 0 else 0

                # score matmuls for group g: chunks (2g, 2g+1)
                def emit_scores(g):
                    st = scs.tile([128, 2, TB], f32, tag="sT")
                    for u in range(2):
                        c = 2 * g + u
                        o = c_off(c)
                        s0 = base + 128 * c
                        nc.tensor.matmul(
                            st[:, u, o:TB],
                            lhsT=kT[:, s0 : s0 + 128],
                            rhs=xT[:, t0 + o : t0 + TB],
                            start=True, stop=True)
                    return st

                sts = {0: emit_scores(0)}
                for g in range(ngroups):
                    if g + 1 < ngroups:
                        sts[g + 1] = emit_scores(g + 1)
                    st = sts.pop(g)
                    om = c_off(2 * g)  # min offset of the two chunks
                    ex = work.tile([128, 2, TB], bf16, tag="exp")
                    nc.scalar.activation(ex[:, :, om:TB], st[:, :, om:TB], Act.Exp)
                    for u in range(2):
                        c = 2 * g + u
                        o = c_off(c)
                        if c >= 4 * j:  # diagonal chunk: mask its 128-col edge
                            nc.vector.tensor_mul(
                                ex[:, u, o : o + 128], ex[:, u, o : o + 128], tri[:])
                        nc.tensor.matmul(
                            outT[:, o:TB],
                            lhsT=v2[:, 16 * b + c, :],
                            rhs=ex[:, u, o:TB],
                            start=(c == 0), stop=(c == nchunks - 1))

                # drain: normalize + transpose to [t, d], ship to rs_all
                oc = drn.tile([D + 1, TB], bf16, tag="oc")
                nc.vector.tensor_copy(oc[:], outT[:])
                tp = psm.tile([128, 4, D + 2], bf16, tag="small")
                for u in range(4):
                    nc.tensor.transpose(
                        tp[:, u, : D + 1], oc[:, 128 * u : 128 * (u + 1)],
                        identb[: D + 1, : D + 1])
                denf = drn.tile([128, 4, 1], f32, tag="denf")
                nc.vector.tensor_copy(denf[:], tp[:, :, D : D + 1])
                recb = drn.tile([128, 4, 1], f32, tag="rec")
                nc.vector.reciprocal_approx_fast(recb[:], denf[:])
                part = drn.tile([128, 4, D], bf16, tag="part")
                nc.vector.tensor_tensor(
                    part[:], tp[:, :, :D],
                    recb.to_broadcast((128, 4, D)), Alu.mult)
                if b < 2:
                    seg, row = rs_a_in, T * b + TB * j
                elif b == 2:
                    seg, row = rs_b_in, TB * j
                elif j < 2:
                    seg, row = rs_c_in, TB * j
                else:
                    seg, row = rs_d_in, TB * (j - 2)
                nc.sync.dma_start(
                    seg[row : row + TB, :]
                    .rearrange("(u p) d -> p u d", p=128),
                    part[:])

            def emit_rs(seg, outt):
                if single:
                    n = outt.shape[0]
                    nc.sync.dma_start(outt[:], seg[:n, :])
                else:
                    nc.gpsimd.collective_compute(
                        "ReduceScatter", Alu.add,
                        replica_groups=[list(range(8))],
                        ins=[seg], outs=[outt[:]])

            def emit_stage_block(q0, nq, rsb):
                """Epilogue for nq*128 tokens (shard rows 128*q0 onward)."""
                def ln(zin, dst, g, be):
                    mt = ep.tile([128, nq, 1], f32, tag="mt")
                    nc.vector.tensor_reduce(mt[:], zin[:], mybir.AxisListType.X,
                                            Alu.add)
                    zc = ep.tile([128, nq, D], f32, tag="zc")
                    # zc = mu - z  (sign folded into g on host)
                    nc.vector.scalar_tensor_tensor(
                        zc[:], mt.to_broadcast(zin.shape), 1.0 / D, zin[:],
                        Alu.mult, Alu.subtract)
                    sq = ep.tile([128, nq, D], f32, tag="sq")
                    nc.vector.tensor_mul(sq[:], zc[:], zc[:])
                    vt = ep.tile([128, nq, 1], f32, tag="vt")
                    nc.vector.tensor_reduce(vt[:], sq[:], mybir.AxisListType.X,
                                            Alu.add)
                    sd = ep.tile([128, nq, 1], f32, tag="sd")
                    nc.scalar.activation(sd[:, :, 0], vt[:, :, 0], Act.Sqrt,
                                         bias=epst[:], scale=1.0 / D)
                    rc = ep.tile([128, nq, 1], f32, tag="rc")
                    nc.vector.reciprocal_approx_fast(rc[:], sd[:])
                    nc.vector.tensor_tensor(
                        zc[:], zc[:], rc.to_broadcast(zc.shape), Alu.mult)
                    nc.vector.tensor_tensor(
                        zc[:], zc[:], g[:, None, :].to_broadcast(zc.shape),
                        Alu.mult)
                    nc.vector.tensor_tensor(
                        dst[:], zc[:], be[:, None, :].to_broadcast(zc.shape),
                        Alu.add)

                rtt = ep.tile([128, nq, D], bf16, tag="rt")
                nc.sync.dma_start(
                    rtt[:], rsb[:].rearrange("(q p) d -> p q d", p=128))
                zt = ep.tile([128, nq, D], f32, tag="zt")
                nc.vector.tensor_tensor(
                    zt[:], xs_all[:, q0 : q0 + nq, :], rtt[:], Alu.add)
                nc.vector.tensor_tensor(
                    zt[:], zt[:], c8bc[:, None, :].to_broadcast(zt.shape),
                    Alu.add)
                h1s = h1b[:, q0 : q0 + nq, :]
                ln(zt, h1s, g1bc, be1bc)
                # h1T slice via PE transposes
                tpE = psm.tile([D, nq, 128], bf16, tag="small")
                for q in range(nq):
                    nc.tensor.transpose(tpE[:, q, :], h1s[:, q, :], identb[:])
                nc.vector.tensor_copy(
                    h1T[:D, 128 * q0 : 128 * (q0 + nq)]
                    .rearrange("p (a c) -> p a c", a=nq), tpE[:])
                # FFN up + relu (relu is resident in every act table)
                f1 = ep.tile([128, 2, nq * 128], bf16, tag="f1")
                for fc in range(2):
                    for s0 in range(0, nq * 128, 512):
                        sw = min(512, nq * 128 - s0)
                        up = psm.tile([128, 512], f32, tag="small")
                        nc.tensor.matmul(
                            up[:, :sw],
                            lhsT=w1a[:, 128 * fc : 128 * (fc + 1)],
                            rhs=h1T[:, 128 * q0 + s0 : 128 * q0 + s0 + sw],
                            start=True, stop=True)
                        nc.scalar.activation(
                            f1[:, fc, s0 : s0 + sw], up[:, :sw], Act.Relu)
                # FFN down into psum: b2 + h1 + relu(h1W1+b1)W2, then LN2
                dn = psm.tile([128, nq, D], f32, tag="small")
                for q in range(nq):
                    nc.tensor.matmul(dn[:, q, :], lhsT=one128[:], rhs=b2r[:],
                                     start=True, stop=False)
                    nc.tensor.matmul(dn[:, q, :], lhsT=identb[:],
                                     rhs=h1s[:, q, :], start=False, stop=False)
                    for fc in range(2):
                        nc.tensor.matmul(
                            dn[:, q, :],
                            lhsT=f1[:, fc, 128 * q : 128 * (q + 1)],
                            rhs=w2[:, fc, :],
                            start=False, stop=(fc == 1))
                o_st = ep.tile([128, nq, D], f32, tag="ot")
                ln(dn, o_st, g2bc, be2bc)
                nc.sync.dma_start(
                    out_d[128 * q0 : 128 * (q0 + nq), :]
                    .rearrange("(q p) d -> p q d", p=128),
                    o_st[:])

            # ---- schedule ----
            # Attention first; RS_A (batches 0-2) fires after batch 2, RS_B
            # (batch 3) at the end. All epilogue stages are emitted after the
            # attention so RS-gated DMAs never block engine queues mid-run.
            for i in range(NJB):
                emit_kg(0, i)
                emit_v2(0, i)
            for b in range(B):
                for j in range(NJB):
                    emit_jblock(b, j)
                    if b == 3 and j == 1:
                        emit_rs(rs_c_in, rs_out_c)
                    if b + 1 < B:
                        if j == 0:
                            emit_kg(b + 1, 0), emit_kg(b + 1, 1)
                        elif j == 1:
                            emit_kg(b + 1, 2), emit_kg(b + 1, 3)
                        elif j == 2:
                            emit_v2(b + 1, 0), emit_v2(b + 1, 1)
                        else:
                            emit_v2(b + 1, 2), emit_v2(b + 1, 3)
                if b == 1:
                    emit_rs(rs_a_in, rs_out_a)
                elif b == 2:
                    emit_rs(rs_b_in, rs_out_b)
                elif b == 3:
                    emit_rs(rs_d_in, rs_out_d)
            # pin epilogue stages to the end of the scheduled engine
            # programs: the Tile scheduler's sim underestimates collective
            # latency and otherwise hoists RS-gated stage ops ahead of
            # batch-3 attention, wedging every engine queue mid-run.
            with tc.tile_wait_until(1.0):
                emit_stage_block(0, 4, rs_out_a)
            with tc.tile_wait_until(1.01):
                emit_stage_block(4, 2, rs_out_b)
            with tc.tile_wait_until(1.02):
                emit_stage_block(6, 1, rs_out_c)
            with tc.tile_wait_until(1.03):
                emit_stage_block(7, 1, rs_out_d)

    nc.compile()
    return nc


def _prep_inputs(inputs, Wq, bq, Wk, bk, Wv, bv, Wp, bp, W1, b1, W2, b2,
                 g1, be1, g2, be2):
    """Host-side input prep: folded per-head weights + per-core maps."""
    import ml_dtypes

    BF16 = ml_dtypes.bfloat16
    x = np.ascontiguousarray(np.asarray(inputs, dtype=F32).reshape(NTOK, D))
    Wq, bq = np.asarray(Wq, np.float64), np.asarray(bq, np.float64)
    Wk, bk = np.asarray(Wk, np.float64), np.asarray(bk, np.float64)
    Wv, bv = np.asarray(Wv, np.float64), np.asarray(bv, np.float64)
    Wp, bp = np.asarray(Wp, np.float64), np.asarray(bp, np.float64)

    bc = lambda v: np.ascontiguousarray(
        np.broadcast_to(np.asarray(v, F32).reshape(1, D), (128, D)))
    bcb = lambda a: np.ascontiguousarray(np.asarray(a, F32).astype(BF16))
    tri = np.triu(np.ones((128, 128), F32)).astype(BF16)
    identb = np.eye(128, dtype=F32).astype(BF16)

    xTa = np.concatenate([x.T, np.ones((1, NTOK), F32)], axis=0).astype(BF16)
    xTa = np.ascontiguousarray(xTa)

    C = sum(bv[h] @ Wp[D * h : D * (h + 1)] for h in range(H)) + bp

    common = dict(
        xT=xTa, tri=tri, identb=identb,
        one128=np.ones((1, 128), F32).astype(BF16),
        b2r=bcb(np.asarray(b2, F32).reshape(1, D)),
        w1a=bcb(np.concatenate(
            [np.asarray(W1, F32), np.asarray(b1, F32).reshape(1, FF)], axis=0)),
        w2=bcb(np.asarray(W2, F32)),
        c8bc=bc(C.astype(F32)),
        g1bc=bc(-np.asarray(g1, F32)), be1bc=bc(be1),
        g2bc=bc(-np.asarray(g2, F32)), be2bc=bc(be2),
    )

    in_maps = []
    for h in range(H):
        # kG weights: wkg[d, r<64] = (Wq Wk^T)[r, d]; wkg[:, 64] = Wk @ bq
        G = Wq[h] @ Wk[h].T
        c = Wk[h] @ bq[h]
        wkg = np.concatenate([G.T, c.reshape(D, 1)], axis=1)  # [64, 65]
        # V path: wvv[:64, :64] = Wv @ Wp_h; ones column via xT ones row
        wvp = Wv[h] @ Wp[D * h : D * (h + 1)]
        wvv = np.zeros((D + 1, D + 1), np.float64)
        wvv[:D, :D] = wvp
        wvv[D, D] = 1.0
        # this core's token shard, per RS segment
        xs_h = np.concatenate(
            [x[512 * h : 512 * (h + 1)],
             x[4096 + 256 * h : 4096 + 256 * (h + 1)],
             x[6144 + 128 * h : 6144 + 128 * (h + 1)],
             x[7168 + 128 * h : 7168 + 128 * (h + 1)]])
        in_maps.append(dict(
            common,
            xs=np.ascontiguousarray(xs_h),
            wkg=np.ascontiguousarray(wkg.astype(F32).astype(BF16)),
            wvv=np.ascontiguousarray(wvv.astype(F32).astype(BF16)),
        ))
    return in_maps


def _gather(results) -> np.ndarray:
    """Reassemble per-core output shards into the full [NTOK, D] output."""
    out = np.empty((NTOK, D), F32)
    for c in range(8):
        shard = results[c]["out"]
        out[512 * c : 512 * (c + 1)] = shard[:512]
        out[4096 + 256 * c : 4096 + 256 * (c + 1)] = shard[512:768]
        out[6144 + 128 * c : 6144 + 128 * (c + 1)] = shard[768:896]
        out[7168 + 128 * c : 7168 + 128 * (c + 1)] = shard[896:]
    return out


def _get_nc():
    if "nc" not in _CACHE:
        _CACHE["nc"] = _build_nc()
    return _CACHE["nc"]


def kernel(**inputs) -> np.ndarray:
    from concourse.bass_utils import run_bass_kernel_spmd

    in_maps = _prep_inputs(**inputs)
    nc = _get_nc()
    res = run_bass_kernel_spmd(nc, in_maps, list(range(8)))
    return _gather(res.results).reshape(B, T, D)
